# revision 1
# baseline (speedup 1.0000x reference)
"""Trainium2 Bass kernel for nn_BlurModel (5x5 box blur -> per-fragment
threshold bisection -> morphological close), distributed over 8 NeuronCores.

Strategy:
  - Shard the 4096x4096 image into 8 row-bands of 512 rows (one fragment-row
    per core), with zero-padded halos supplied by the host.
  - Phase 1 (device): separable 5x5 box sum. Horizontal via shifted DVE adds
    (exact f32), vertical via PE fp32 matmul with a 0/1 banded matrix
    (exact: weights 1.0, fp32 H/L split reconstructs inputs exactly),
    final 1/25 scale on ACT. Output: conved image.
  - Host: per-fragment threshold walk replicated with exact float32
    semantics (sort + binary search for exact counts) - matches the
    reference's lax.while_loop bit-for-bit given the same conv values.
  - Phase 2 (device): mask = sign(conved - th) in {-1,+1} (ACT), then
    morphological close via 5x5 window *sums*: dilated = sign(sum + 24)
    (any +1 in window <=> sum > -24), eroded = (sum(dil) > 10*nvalid-25.5)
    (all valid cells +1). The 2D window sums run on the PE (3 accumulating
    bf16 matmuls: banded vertical matrix x horizontally pair-summed
    operands). Out-of-image rows handled by per-partition biases, columns
    by -1/+1 borders; the 16 corner pixels where both interact are patched
    on the host.
"""
import os
import numpy as np
import ml_dtypes
from contextlib import ExitStack

import concourse.bacc as bacc
import concourse.tile as tile
import concourse.mybir as mybir
from concourse.bass_utils import run_bass_kernel_spmd

F32 = mybir.dt.float32
BF16 = mybir.dt.bfloat16
AOP = mybir.AluOpType
AFT = mybir.ActivationFunctionType

H = W = 4096
SF = 8
K = H // SF          # 512 fragment side
NCORES = 8
BAND = H // NCORES   # 512 rows per core
PAD = 4              # halo rows on each side of a band
BROWS = BAND + 2 * PAD  # 520
NFRAG = K * K        # 262144 pixels per fragment

STEP = np.float32(0.0005)
UP_TH = np.float32(0.1 + 0.02)
DN_TH = np.float32(0.1 - 0.02)
TH_INIT = np.float32(0.5)

# phase-1 vertical chunks: (out_row_start_rel_to_band, n_out_rows)
P1_CHUNKS = [(0, 124), (124, 124), (248, 124), (372, 124)]
# phase-2 chunks: (out_row_start, n_out_rows, psum_row_lo) - last chunk
# recomputes rows already written and stores only partitions [lo, 120)
P2_CHUNKS = [(0, 120, 0), (120, 120, 0), (240, 120, 0), (360, 120, 0),
             (392, 120, 88)]

_CACHE = {}

LAST_RESULTS = []  # BassKernelResults of the most recent kernel() call


def _build_phase1():
    nc = bacc.Bacc("TRN2", target_bir_lowering=False, debug=False,
                   enable_asserts=False, num_devices=NCORES)
    xb = nc.dram_tensor("xb", [BROWS, W], F32, kind="ExternalInput").ap()
    bmain = nc.dram_tensor("bmain", [128, 124], F32, kind="ExternalInput").ap()
    blast = nc.dram_tensor("blast", [116, 16], F32, kind="ExternalInput").ap()
    scl = nc.dram_tensor("scl", [128, 1], F32, kind="ExternalInput").ap()
    cb = nc.dram_tensor("cb", [BAND, W], F32, kind="ExternalOutput").ap()

    with tile.TileContext(nc) as tc:
        with ExitStack() as ctx:
            const = ctx.enter_context(tc.tile_pool(name="const", bufs=1))
            io = ctx.enter_context(tc.tile_pool(name="io", bufs=2))
            op = ctx.enter_context(tc.tile_pool(name="op", bufs=2))
            work = ctx.enter_context(tc.tile_pool(name="work", bufs=2))
            tmp = ctx.enter_context(tc.tile_pool(name="tmp", bufs=2))
            sm = ctx.enter_context(tc.tile_pool(name="sm", bufs=1))
            pp = ctx.enter_context(tc.tile_pool(name="pp", bufs=2, space="PSUM"))

            b_main = const.tile([128, 124], F32)
            nc.scalar.dma_start(b_main[:], bmain)
            b_last = const.tile([116, 16], F32)
            nc.scalar.dma_start(b_last[:], blast)
            scl_sb = const.tile([128, 1], F32)
            nc.scalar.dma_start(scl_sb[:], scl)

            # --- last chunk: out rows 496..511 from band rows 498..517,
            # packed as 4 column-slabs of 20 rows to keep DVE work small ---
            SW = 1024
            xt4 = sm.tile([116, SW + 4], F32, tag="xt4")
            # slab s lives at partitions [32s, 32s+20);
            # tile col f of slab s holds image col 1024*s + f - 2
            nc.any.memset(xt4[0:20, 0:2], 0.0)            # slab 0: cols -2,-1
            nc.any.memset(xt4[96:116, SW + 2:SW + 4], 0.0)  # slab 3: 4096,4097
            for s in range(4):
                f0 = 2 if s == 0 else 0
                f1 = SW + 2 if s == 3 else SW + 4
                c0 = SW * s - 2 + f0
                nc.scalar.dma_start(xt4[32 * s:32 * s + 20, f0:f1],
                                    xb[498:518, c0:c0 + (f1 - f0)])
            xts, t2s, hbs = {}, {}, {}

            def emit_load(ci):
                r, m = P1_CHUNKS[ci]
                in_lo = r + 2
                xt = io.tile([128, W + 4], F32, tag="xt", name=f"xt{ci}")
                nc.any.memset(xt[:, 0:2], 0.0)
                nc.any.memset(xt[:, W + 2:W + 4], 0.0)
                if ci == 0:
                    # finer first load so the very first matmul's input
                    # chain (t1->t2->hb over cols 0..1024) is ready asap
                    nc.sync.dma_start(xt[:, 2:1030],
                                      xb[in_lo:in_lo + 128, 0:1028])
                    nc.sync.dma_start(xt[:, 1030:HW2 + 4],
                                      xb[in_lo:in_lo + 128, 1028:HW2 + 2])
                else:
                    nc.sync.dma_start(xt[:, 2:HW2 + 4],
                                      xb[in_lo:in_lo + 128, 0:HW2 + 2])
                nc.sync.dma_start(xt[:, HW2 + 4:W + 2],
                                  xb[in_lo:in_lo + 128, HW2 + 2:W])
                xts[ci] = xt

            def emit_t12(ci, half):
                xt = xts[ci]
                if half == 0:
                    t1 = tmp.tile([128, W + 3], F32, tag="t1", name=f"t1_{ci}")
                    t2 = tmp.tile([128, W + 1], F32, tag="t2", name=f"t2_{ci}")
                    t2s[ci] = (t1, t2)
                    if ci == 0:
                        nc.vector.tensor_add(t1[:, 0:1029], xt[:, 0:1029],
                                             xt[:, 1:1030])
                        nc.vector.tensor_add(t2[:, 0:1027], t1[:, 0:1027],
                                             t1[:, 2:1029])
                        nc.vector.tensor_add(hbs[0][:, 0:1024],
                                             t2[:, 0:1024], xt[:, 4:1028])
                        nc.vector.tensor_add(t1[:, 1029:HW2 + 2],
                                             xt[:, 1029:HW2 + 2],
                                             xt[:, 1030:HW2 + 3])
                        nc.vector.tensor_add(t2[:, 1027:HW2],
                                             t1[:, 1027:HW2],
                                             t1[:, 1029:HW2 + 2])
                        return
                    nc.vector.tensor_add(t1[:, 0:HW2 + 2], xt[:, 0:HW2 + 2],
                                         xt[:, 1:HW2 + 3])
                    nc.vector.tensor_add(t2[:, 0:HW2], t1[:, 0:HW2],
                                         t1[:, 2:HW2 + 2])
                else:
                    t1, t2 = t2s[ci]
                    nc.vector.tensor_add(t1[:, HW2 + 2:W + 3],
                                         xt[:, HW2 + 2:W + 3],
                                         xt[:, HW2 + 3:W + 4])
                    nc.vector.tensor_add(t2[:, HW2:W + 1],
                                         t1[:, HW2:W + 1],
                                         t1[:, HW2 + 2:W + 3])

            def emit_hbq(ci, z):
                xt = xts[ci]
                t2 = t2s[ci][1]
                if z == 0:
                    if ci == 0:
                        return  # cols 0:1024 already produced in emit_t12
                    hbs[ci] = work.tile([128, W], F32, tag="hb", name=f"hb{ci}")
                hb = hbs[ci]
                a, b = 1024 * z, 1024 * (z + 1)
                nc.vector.tensor_add(hb[:, a:b], t2[:, a:b],
                                     xt[:, a + 4:b + 4])

            def emit_mm(ci, two=False):
                r, m = P1_CHUNKS[ci]
                out = op.tile([m, W], F32, tag="out", name=f"out{ci}")
                for q in range(W // 2048):
                    ps = pp.tile([m, 2048], F32, tag="ps", name=f"ps{ci}_{q}")
                    for sblk in range(4):
                        c0 = 2048 * q + 512 * sblk
                        po = ps[:, 512 * sblk:512 * (sblk + 1)]
                        if two:
                            # fold the last horizontal term into a second
                            # accumulating matmul (keeps PE warm on the
                            # DVE-light chunks)
                            t2 = t2s[ci][1]
                            xt = xts[ci]
                            nc.tensor.matmul(po, b_main[:, 0:m],
                                             t2[:, c0:c0 + 512],
                                             start=True, stop=False)
                            nc.tensor.matmul(po, b_main[:, 0:m],
                                             xt[:, c0 + 4:c0 + 516],
                                             start=False, stop=True)
                        else:
                            nc.tensor.matmul(po, b_main[:, 0:m],
                                             hbs[ci][:, c0:c0 + 512],
                                             start=True, stop=True)
                    nc.scalar.activation(out[:, 2048 * q:2048 * (q + 1)],
                                         ps[:], AFT.Copy, bias=0.0,
                                         scale=scl_sb[0:m, 0:1])
                    nc.gpsimd.dma_start(cb[r:r + m, 2048 * q:2048 * (q + 1)],
                                        out[:, 2048 * q:2048 * (q + 1)])

            HW2 = W // 2
            NCH = len(P1_CHUNKS)
            TWO_MM = (2,)
            emit_load(0)
            hbs[0] = work.tile([128, W], F32, tag="hb", name="hb0")
            emit_t12(0, 0)
            for ci in range(NCH):
                nxt = ci + 1 if ci + 1 < NCH else None
                if nxt is not None:
                    emit_load(nxt)
                if ci in TWO_MM:
                    emit_t12(ci, 1)
                    if nxt is not None:
                        emit_t12(nxt, 0)
                else:
                    # interleave this chunk's hb quarters with the remaining
                    # adds so PE matmul inputs arrive evenly spaced
                    emit_hbq(ci, 0)
                    emit_t12(ci, 1)
                    emit_hbq(ci, 1)
                    emit_hbq(ci, 2)
                    if nxt is not None:
                        emit_t12(nxt, 0)
                    emit_hbq(ci, 3)
                emit_mm(ci, two=ci in TWO_MM)
                if ci == 0:
                    t14 = sm.tile([116, SW + 3], F32, tag="t14")
                    nc.vector.tensor_add(t14[:, 0:SW + 3], xt4[:, 0:SW + 3],
                                         xt4[:, 1:SW + 4])
                    t24 = sm.tile([116, SW + 1], F32, tag="t24")
                    nc.vector.tensor_add(t24[:, 0:SW + 1], t14[:, 0:SW + 1],
                                         t14[:, 2:SW + 3])
                    hb4 = sm.tile([116, SW], F32, tag="hb4")
                    nc.vector.tensor_add(hb4[:, 0:SW], t24[:, 0:SW],
                                         xt4[:, 4:SW + 4])
                    out4 = sm.tile([16, W], F32, tag="out4")
                    for q4 in range(2):
                        ps4 = pp.tile([16, 2048], F32, tag="ps",
                                      name=f"ps4_{q4}")
                        for s4 in range(4):
                            n4 = 4 * q4 + s4   # 512-col block index 0..7
                            sl = n4 // 2       # slab
                            lc = 512 * (n4 % 2)  # local col in slab
                            nc.tensor.matmul(
                                ps4[:, 512 * s4:512 * (s4 + 1)],
                                b_last[32 * sl:32 * sl + 20, 0:16],
                                hb4[32 * sl:32 * sl + 20, lc:lc + 512],
                                start=True, stop=True,
                                tile_position=(96, 0) if sl == 3 else None)
                        nc.scalar.activation(
                            out4[:, 2048 * q4:2048 * (q4 + 1)], ps4[:],
                            AFT.Copy, bias=0.0, scale=scl_sb[0:16, 0:1])
                    nc.gpsimd.dma_start(cb[496:512, :], out4[:])

    nc.compile()
    return nc


def _build_phase2():
    nc = bacc.Bacc("TRN2", target_bir_lowering=False, debug=False,
                   enable_asserts=False, num_devices=NCORES)
    cbp = nc.dram_tensor("cbp", [BROWS, W], F32, kind="ExternalInput").ap()
    nthv = nc.dram_tensor("nthv", [128, 40], F32, kind="ExternalInput").ap()
    ndthr = nc.dram_tensor("ndthr", [128, 5], F32, kind="ExternalInput").ap()
    tvec = nc.dram_tensor("tvec", [128, 5], F32, kind="ExternalInput").ap()
    b5a = nc.dram_tensor("b5a", [128, 124], BF16, kind="ExternalInput").ap()
    b5b = nc.dram_tensor("b5b", [124, 120], BF16, kind="ExternalInput").ap()
    ob = nc.dram_tensor("ob", [BAND, W], BF16, kind="ExternalOutput").ap()

    with tile.TileContext(nc) as tc:
        with ExitStack() as ctx:
            const = ctx.enter_context(tc.tile_pool(name="const", bufs=1))
            io = ctx.enter_context(tc.tile_pool(name="io", bufs=3))
            work = ctx.enter_context(tc.tile_pool(name="work", bufs=2))
            pp1 = ctx.enter_context(tc.tile_pool(name="pp1", bufs=2,
                                                 space="PSUM"))
            pp2 = ctx.enter_context(tc.tile_pool(name="pp2", bufs=2,
                                                 space="PSUM"))

            b5a_sb = const.tile([128, 124], BF16)
            nc.sync.dma_start(b5a_sb[:], b5a)
            b5b_sb = const.tile([124, 120], BF16)
            nc.sync.dma_start(b5b_sb[:], b5b)
            nthv_sb = const.tile([128, 40], F32)
            nc.sync.dma_start(nthv_sb[:], nthv)
            ndthr_sb = const.tile([128, 5], F32)
            nc.sync.dma_start(ndthr_sb[:], ndthr)
            tvec_sb = const.tile([128, 5], F32)
            nc.sync.dma_start(tvec_sb[:], tvec)

            # warm up the Sign activation table before any data arrives
            warm = const.tile([1, 1], BF16)
            nc.scalar.activation(warm[:], tvec_sb[0:1, 0:1], AFT.Sign,
                                 bias=0.0)

            for ci, (r, m, plo) in enumerate(P2_CHUNKS):
                HW2 = W // 2
                ct = io.tile([128, W], F32, tag="ct")
                nq = 8 if ci == 0 else 2
                for z in range(nq):
                    qa, qb = W * z // nq, W * (z + 1) // nq
                    nc.sync.dma_start(ct[:, qa:qb], cbp[r:r + 128, qa:qb])

                # mask = sign(conved - th) in {-1,+1} bf16, -1 borders
                mt = work.tile([128, W + 4], BF16, tag="mt")
                nc.any.memset(mt[:, 0:2], -1.0)
                nc.any.memset(mt[:, W + 2:W + 4], -1.0)
                for n in range(8):
                    nc.scalar.activation(
                        mt[:, 2 + 512 * n:2 + 512 * (n + 1)],
                        ct[:, 512 * n:512 * (n + 1)], AFT.Sign,
                        bias=nthv_sb[:, 8 * ci + n:8 * ci + n + 1])

                # horizontal pair sums of the mask
                mp = work.tile([128, W + 3], BF16, tag="mp")
                for z in range(4):
                    a = 1028 * z
                    b = min(1028 * (z + 1), W + 3)
                    nc.vector.tensor_add(mp[:, a:b], mt[:, a:b],
                                         mt[:, a + 1:b + 1])

                # dilation: 5x5 sum via 3 accumulating matmuls;
                # dil = sign(sum + 24) (or forced -1 off-image), +1 borders
                dil = work.tile([124, W + 4], BF16, tag="dil")
                nc.any.memset(dil[:, 0:2], 1.0)
                nc.any.memset(dil[:, W + 2:W + 4], 1.0)
                for q in range(4):
                    ps = pp1.tile([124, 1024], F32, tag="ps1")
                    for s in range(2):
                        c0 = 1024 * q + 512 * s
                        po = ps[:, 512 * s:512 * (s + 1)]
                        nc.tensor.matmul(po, b5a_sb[:, 0:124],
                                         mp[:, c0:c0 + 512],
                                         start=True, stop=False)
                        nc.tensor.matmul(po, b5a_sb[:, 0:124],
                                         mp[:, c0 + 2:c0 + 514],
                                         start=False, stop=False)
                        nc.tensor.matmul(po, b5a_sb[:, 0:124],
                                         mt[:, c0 + 4:c0 + 516],
                                         start=False, stop=True)
                    nc.scalar.activation(
                        dil[:, 2 + 1024 * q:2 + 1024 * (q + 1)], ps[:],
                        AFT.Sign, bias=ndthr_sb[0:124, ci:ci + 1])

                # horizontal pair sums of dilated
                mp2 = work.tile([124, W + 3], BF16, tag="mp2")
                for z in range(4):
                    a = 1028 * z
                    b = min(1028 * (z + 1), W + 3)
                    nc.vector.tensor_add(mp2[:, a:b], dil[:, a:b],
                                         dil[:, a + 1:b + 1])

                # erosion: 5x5 sum of dilated; out = sum > 10*nvalid-25.5
                out = io.tile([120, W], BF16, tag="out")
                for q in range(4):
                    ps2 = pp2.tile([120, 1024], F32, tag="ps2")
                    for s in range(2):
                        c0 = 1024 * q + 512 * s
                        po = ps2[:, 512 * s:512 * (s + 1)]
                        nc.tensor.matmul(po, b5b_sb[:, 0:120],
                                         mp2[:, c0:c0 + 512],
                                         start=True, stop=False)
                        nc.tensor.matmul(po, b5b_sb[:, 0:120],
                                         mp2[:, c0 + 2:c0 + 514],
                                         start=False, stop=False)
                        nc.tensor.matmul(po, b5b_sb[:, 0:120],
                                         dil[:, c0 + 4:c0 + 516],
                                         start=False, stop=True)
                    nc.vector.tensor_scalar(
                        out[:, 1024 * q:1024 * (q + 1)], ps2[:],
                        tvec_sb[0:120, ci:ci + 1], None, AOP.is_gt)
                nc.gpsimd.dma_start(ob[r + plo:r + m, 0:HW2],
                                  out[plo:120, 0:HW2])
                nc.gpsimd.dma_start(ob[r + plo:r + m, HW2:W],
                                  out[plo:120, HW2:W])
    nc.compile()
    return nc


def _get(name, builder):
    if name not in _CACHE:
        _CACHE[name] = builder()
    return _CACHE[name]


def _run_spmd(nc, in_maps, trace):
    """run_bass_kernel_spmd with one retry (axon RPC can fail transiently)."""
    import time as _time
    last = None
    for attempt in range(3):
        try:
            return run_bass_kernel_spmd(nc, in_maps,
                                        core_ids=list(range(NCORES)),
                                        trace=trace)
        except Exception as e:  # noqa: BLE001 - retry any transport error
            last = e
            _time.sleep(2.0 * (attempt + 1))
    raise last


def _band_matrices():
    bmain = np.zeros((128, 124), np.float32)
    for j in range(124):
        bmain[j:j + 5, j] = 1.0
    blast = np.zeros((116, 16), np.float32)
    for sl in range(4):
        for j in range(16):
            blast[32 * sl + j:32 * sl + j + 5, j] = 1.0
    b5a = np.zeros((128, 124), ml_dtypes.bfloat16)
    for j in range(124):
        b5a[j:j + 5, j] = 1.0
    b5b = np.zeros((124, 120), ml_dtypes.bfloat16)
    for j in range(120):
        b5b[j:j + 5, j] = 1.0
    return bmain, blast, b5a, b5b


def _pad_band(img, c):
    """rows [512c-4, 512c+516) of img, zero-padded outside [0, H)."""
    out = np.zeros((BROWS, W), np.float32)
    lo = BAND * c - PAD
    hi = BAND * c + BAND + PAD
    slo, shi = max(lo, 0), min(hi, H)
    out[slo - lo:shi - lo, :] = img[slo:shi, :]
    return out


def host_walk(conved):
    """Exact replication of the reference threshold walk (float32)."""
    frags = (conved.reshape(SF, K, SF, K).transpose(0, 2, 1, 3)
             .reshape(64, NFRAG))
    srt = np.sort(frags, axis=1)
    ths = np.empty(64, np.float32)
    th = TH_INIT
    inv_n = 1.0 / NFRAG  # NFRAG = 2^18 -> exact scaling
    for i in range(64):
        s = srt[i]
        while True:
            cnt = NFRAG - np.searchsorted(s, th, side='right')
            if not (np.float32(cnt * inv_n) < UP_TH):
                break
            th = np.float32(th - STEP)
        while True:
            cnt = NFRAG - np.searchsorted(s, th, side='right')
            if not (np.float32(cnt * inv_n) > DN_TH):
                break
            th = np.float32(th + STEP)
        ths[i] = th
    return ths


def _close_patch(mask, rows, cols):
    """Valid-window 5x5 close of `mask` evaluated at the given global
    (row, col) pixels. mask is the full HxW {0,1} array."""
    out = {}
    for R in rows:
        for C in cols:
            ero = 1.0
            for dr in range(-2, 3):
                for dc in range(-2, 3):
                    rr, cc = R + dr, C + dc
                    if not (0 <= rr < H and 0 <= cc < W):
                        continue
                    # dilated at (rr, cc)
                    r0, r1 = max(rr - 2, 0), min(rr + 2, H - 1)
                    c0, c1 = max(cc - 2, 0), min(cc + 2, W - 1)
                    d = mask[r0:r1 + 1, c0:c1 + 1].max()
                    if d < 0.5:
                        ero = 0.0
                        break
                if ero == 0.0:
                    break
            out[(R, C)] = ero
    return out


def kernel(x, blur_k):
    global LAST_RESULTS
    LAST_RESULTS = []
    x2 = np.ascontiguousarray(np.asarray(x, np.float32).reshape(H, W))
    scale = np.float32(np.asarray(blur_k).reshape(-1)[0])

    bmain, blast, b5a, b5b = _band_matrices()
    trace = bool(int(os.environ.get("BASS_TRACE", "0") or "0"))

    # ---- phase 1: box blur ----
    nc1 = _get("p1", _build_phase1)
    scl = np.full((128, 1), scale, np.float32)
    in_maps = [{"xb": _pad_band(x2, c), "bmain": bmain, "blast": blast,
                "scl": scl} for c in range(NCORES)]
    res1 = _run_spmd(nc1, in_maps, trace)
    LAST_RESULTS.append(res1)
    conved = np.empty((H, W), np.float32)
    for c in range(NCORES):
        conved[BAND * c:BAND * (c + 1), :] = res1.results[c]["cb"]

    # ---- host: exact threshold walk ----
    ths = host_walk(conved)
    th_grid = ths.reshape(SF, SF)  # [fragrow, fragcol]

    # ---- phase 2: threshold + morphological close ----
    nc2 = _get("p2", _build_phase2)
    in_maps2 = []
    for c in range(NCORES):
        nthv = np.empty((128, 40), np.float32)
        ndthr = np.full((128, 5), 24.0, np.float32)
        tvec = np.full((128, 5), 24.5, np.float32)
        for ci, (r, m, plo) in enumerate(P2_CHUNKS):
            for p in range(128):
                row = BAND * c + r - PAD + p
                fr = min(max(row // K, 0), SF - 1)
                for f in range(SF):
                    nthv[p, 8 * ci + f] = -th_grid[fr, f]
            for j in range(124):
                row = BAND * c + r - 2 + j
                if row < 0 or row >= H:
                    ndthr[j, ci] = -1e9
            for j in range(120):
                row = BAND * c + r + j
                if 0 <= row < H:
                    nv = min(row + 2, H - 1) - max(row - 2, 0) + 1
                    tvec[j, ci] = 10 * nv - 25.5
        in_maps2.append({"cbp": _pad_band(conved, c), "nthv": nthv,
                         "ndthr": ndthr, "tvec": tvec,
                         "b5a": b5a, "b5b": b5b})
    res2 = _run_spmd(nc2, in_maps2, trace)
    LAST_RESULTS.append(res2)
    out = np.empty((H, W), np.float32)
    for c in range(NCORES):
        out[BAND * c:BAND * (c + 1), :] = (
            res2.results[c]["ob"].astype(np.float32))

    # ---- host: patch the 16 corner pixels (row-pad x col-border overlap) --
    mask = np.zeros((H, W), np.float32)
    for (R, C) in [(0, 0), (0, W - 8), (H - 8, 0), (H - 8, W - 8)]:
        rs, cs = slice(R, R + 8), slice(C, C + 8)
        sub = conved[rs, cs]
        fr = min(max((R + 4) // K, 0), SF - 1)
        fcs = min(max((C + 4) // K, 0), SF - 1)
        mask[rs, cs] = (sub > th_grid[fr, fcs]).astype(np.float32)
    fixed = _close_patch(mask, [0, 1, H - 2, H - 1], [0, 1, W - 2, W - 1])
    for (R, C), v in fixed.items():
        out[R, C] = v
    return out.reshape(1, 1, H, W)

